# revision 8
# baseline (speedup 1.0000x reference)
"""Bass/Trainium2 kernel for a 2-layer GCN with knowledge-enhanced output
(nn_KeGNN): y = log_softmax(relu(GCN2(relu(GCN1(x))) + P*K*U)).

Distribution strategy (8 NeuronCores, SPMD one NEFF):
  * Nodes are partitioned into 8 contiguous shards (12500 each); core c owns
    the edges whose *destination* is in shard c and produces the output rows
    of its shard.
  * GCN normalization is folded node-wise: with dinv = 1/sqrt(deg),
    table = dinv * (H @ W) gives messages, and the aggregated sum is scaled
    by dinv[dst].  The per-edge segment-sum becomes:
       agg[dst-tile] += S.T @ G        (TensorE matmul, PSUM accumulate)
    where G = dma_gather(table, src-index) and S is a 0/1 selection matrix
    built on VectorE with one is_equal against a static iota row.
  * Layer-1 table (dinv * (x @ W1), all 100k nodes) is computed redundantly
    on every core from a transposed copy of x -- cheaper than collectives.
  * Layer-2 table (dinv * (h1 @ W2), padded 40->64) is computed per-shard and
    AllGathered between the layers.
  * Source indices are int16 (hardware gather limit 32767) so the gather is
    split into 4 source blocks of 25000 nodes; per (dst-tile, block) segments
    are padded to multiples of 128 tokens, identically across cores so one
    program serves all 8 cores (per-core behavior differs only through the
    per-core index/dstloc input arrays).
"""

import numpy as np


# ----------------------------------------------------------------- config --
class CFG:
    N = 100000      # nodes
    F = 128         # input feature dim
    H = 64          # hidden dim
    O = 40          # output dim
    E = 1600000     # edges (without self loops)
    C = 8           # cores
    NBLK = 4        # src blocks (int16 gather index limit)
    CH_KT = 8       # K-tiles (of 128 tokens) per dma_gather call
                    # (SWDGE ucode ring: one call must be <= 1024 descriptors)
    SLAB = 2048     # nodes per xT slab load in table1 build
    DMA_SCRATCH = 16384   # per-partition SWDGE desc-ring carveout bytes
    STG = 14        # dst-tiles per staged DRAM write in postproc

    def __init__(self, **kw):
        for k, v in kw.items():
            setattr(self, k, v)
        assert self.N % self.C == 0
        self.SHARD = self.N // self.C
        self.NT = -(-self.SHARD // 128)          # dst tiles per core
        self.LASTV = self.SHARD - (self.NT - 1) * 128  # valid rows in last tile
        assert self.N % self.NBLK == 0
        self.BLK = self.N // self.NBLK
        assert self.BLK <= 32767
        self.NBT = -(-self.BLK // 128)           # node tiles per block
        self.HP = 64                             # padded layer-2 table width
        assert self.O <= self.HP


def _cdiv(a, b):
    return -(-a // b)


# ----------------------------------------------------- host preprocessing --
def _preprocess(edge_index, cfg: CFG):
    """Partition/sort edges, compute degrees, build per-core gather indices.

    Returns (deg, nk, per_core) where nk[b][t] is the common (cross-core max)
    K-tile count of (src-block b, dst-tile t) and per_core is a list of dicts
    with the wrapped idx/dstloc arrays.
    """
    N, C, NBLK = cfg.N, cfg.C, cfg.NBLK
    NT, SHARD, BLK = cfg.NT, cfg.SHARD, cfg.BLK

    loops = np.arange(N, dtype=np.int64)
    src = np.concatenate([np.asarray(edge_index[0], dtype=np.int64), loops])
    dst = np.concatenate([np.asarray(edge_index[1], dtype=np.int64), loops])
    deg = np.bincount(dst, minlength=N).astype(np.float32)

    core = dst // SHARD
    tloc = (dst % SHARD) // 128
    blk = src // BLK
    key = (core * NBLK + blk) * NT + tloc
    order = np.argsort(key, kind="stable")
    s_src = src[order]
    s_dst = dst[order]
    s_key = key[order]

    ngroups = C * NBLK * NT
    cnt = np.bincount(s_key, minlength=ngroups).reshape(C, NBLK, NT)
    starts = np.zeros(ngroups + 1, dtype=np.int64)
    np.cumsum(cnt.reshape(-1), out=starts[1:])

    nk = _cdiv(cnt.max(axis=0), 128)  # [NBLK, NT]
    nktot = int(nk.sum())
    ntok = nktot * 128

    per_core = []
    for c in range(C):
        idx_stream = np.zeros(ntok, dtype=np.int16)
        dloc_stream = np.full(ntok, 999.0, dtype=np.float32)
        pos = 0
        for b in range(NBLK):
            for t in range(NT):
                g = (c * NBLK + b) * NT + t
                a, e = starts[g], starts[g + 1]
                n = e - a
                idx_stream[pos:pos + n] = (s_src[a:e] - b * BLK).astype(np.int16)
                dloc_stream[pos:pos + n] = (
                    s_dst[a:e] - (c * SHARD + t * 128)
                ).astype(np.float32)
                pos += int(nk[b, t]) * 128
        assert pos == ntok
        idx_rep = np.ascontiguousarray(
            np.tile(idx_stream.reshape(-1, 16).T, (8, 1))
        )  # [128, ntok//16]
        dloc_w = np.ascontiguousarray(
            dloc_stream.reshape(-1, 128).T
        )  # [128, nktot]
        per_core.append({"idx": idx_rep, "dloc": dloc_w})

    return deg, nk, per_core


def _wrap_deg(deg, cfg: CFG):
    """degB [128, NBLK*NBT] (block-wrapped, pad 1.0) and per-core degS
    [128, NT] (shard-wrapped, pad 1.0)."""
    N, NBLK, BLK, NBT = cfg.N, cfg.NBLK, cfg.BLK, cfg.NBT
    C, SHARD, NT = cfg.C, cfg.SHARD, cfg.NT
    degB = np.ones((128, NBLK * NBT), dtype=np.float32)
    for b in range(NBLK):
        for j in range(NBT):
            base = b * BLK + j * 128
            m = min(128, (b + 1) * BLK - base, N - base)
            if m > 0:
                degB[:m, b * NBT + j] = deg[base:base + m]
    degS = np.ones((C, 128, NT), dtype=np.float32)
    for c in range(C):
        for t in range(NT):
            base = c * SHARD + t * 128
            m = min(128, (c + 1) * SHARD - base)
            degS[c, :m, t] = deg[base:base + m]
    return degB, degS


# ------------------------------------------------------------ bass program --
def _build(cfg: CFG, nk):
    import concourse.bacc as bacc
    import concourse.mybir as mybir
    from concourse import tile

    f32 = mybir.dt.float32
    i16 = mybir.dt.int16
    i32 = mybir.dt.int32
    ALU = mybir.AluOpType
    ACTF = mybir.ActivationFunctionType

    N, F, H, O, C = cfg.N, cfg.F, cfg.H, cfg.O, cfg.C
    NBLK, BLK, NBT = cfg.NBLK, cfg.BLK, cfg.NBT
    NT, SHARD, LASTV, HP = cfg.NT, cfg.SHARD, cfg.LASTV, cfg.HP
    CH_KT, SLAB, STG = cfg.CH_KT, cfg.SLAB, cfg.STG

    nktot = int(nk.sum())
    ntok = nktot * 128
    # K-tile offset of each block's stream
    blk_kt_base = [0] * (NBLK + 1)
    for b in range(NBLK):
        blk_kt_base[b + 1] = blk_kt_base[b] + int(nk[b].sum())

    nc = bacc.Bacc("TRN2", target_bir_lowering=False, debug=False,
                   num_devices=cfg.C,
                   dynamic_dma_scratch_size=cfg.DMA_SCRATCH)

    # ---- DRAM I/O
    xT_d = nc.dram_tensor("xT", [F, N], f32, kind="ExternalInput")
    degB_d = nc.dram_tensor("degB", [128, NBLK * NBT], f32, kind="ExternalInput")
    degS_d = nc.dram_tensor("degS", [128, NT], f32, kind="ExternalInput")
    idx_d = nc.dram_tensor("idx", [128, ntok // 16], i16, kind="ExternalInput")
    dloc_d = nc.dram_tensor("dloc", [128, nktot], f32, kind="ExternalInput")
    W1_d = nc.dram_tensor("W1", [F, H], f32, kind="ExternalInput")
    W2_d = nc.dram_tensor("W2", [H, O], f32, kind="ExternalInput")
    b1_d = nc.dram_tensor("b1", [1, H], f32, kind="ExternalInput")
    b2_d = nc.dram_tensor("b2", [1, O], f32, kind="ExternalInput")
    P_d = nc.dram_tensor("P", [1, O], f32, kind="ExternalInput")
    K_d = nc.dram_tensor("K", [1, O], f32, kind="ExternalInput")
    U_d = nc.dram_tensor("U", [1, O], f32, kind="ExternalInput")
    out_d = nc.dram_tensor("out", [SHARD, O], f32, kind="ExternalOutput")

    tab1 = [
        nc.dram_tensor(f"tab1_{b}", [min(BLK, N - b * BLK), H], f32)
        for b in range(NBLK)
    ]
    t2loc = nc.dram_tensor("t2loc", [SHARD, HP], f32)
    tab2 = nc.dram_tensor("tab2", [N, HP], f32, addr_space="Shared")

    with tile.TileContext(nc, num_cores=C) as tc:
        with (
            tc.tile_pool(name="const", bufs=1) as const,
            tc.tile_pool(name="xslab", bufs=2) as xpool,
            tc.tile_pool(name="t1st", bufs=2) as t1pool,
            tc.tile_pool(name="g", bufs=4) as gpool,
            tc.tile_pool(name="s", bufs=4) as spool,
            tc.tile_pool(name="work", bufs=2) as work,
            tc.tile_pool(name="post", bufs=2) as post,
            tc.tile_pool(name="ost", bufs=2) as opool,
            tc.tile_pool(name="ps_seg", bufs=3, space="PSUM") as ps_seg,
            tc.tile_pool(name="ps_bld", bufs=2, space="PSUM") as ps_bld,
            tc.tile_pool(name="ps_tr", bufs=1, space="PSUM") as ps_tr,
            tc.tile_pool(name="ps_t2", bufs=1, space="PSUM") as ps_t2,
        ):
            # ---------------- constants / small inputs
            iota_i = const.tile([128, 128], i32)
            nc.gpsimd.iota(iota_i[:, :], pattern=[[1, 128]], base=0,
                           channel_multiplier=0)
            IOTA = const.tile([128, 128], f32)
            nc.vector.tensor_copy(IOTA[:, :], iota_i[:, :])
            pidx_i = const.tile([128, 1], i32)
            nc.gpsimd.iota(pidx_i[:, :], pattern=[[0, 1]], base=0,
                           channel_multiplier=1)
            PIDX = const.tile([128, 1], f32)
            nc.vector.tensor_copy(PIDX[:, :], pidx_i[:, :])
            ID = const.tile([128, 128], f32)
            nc.vector.tensor_scalar(out=ID[:, :], in0=IOTA[:, :],
                                    scalar1=PIDX[:, :], scalar2=None,
                                    op0=ALU.is_equal)

            W1s = const.tile([F, H], f32)
            nc.sync.dma_start(W1s[:, :], W1_d[:, :])
            W2s = const.tile([H, O], f32)
            nc.sync.dma_start(W2s[:, :], W2_d[:, :])

            b1row = const.tile([1, H], f32)
            nc.sync.dma_start(b1row[:, :], b1_d[:, :])
            BIAS1 = const.tile([128, H], f32)
            nc.gpsimd.partition_broadcast(BIAS1[:, :], b1row[:, :])

            b2row = const.tile([1, O], f32)
            nc.sync.dma_start(b2row[:, :], b2_d[:, :])
            prow = const.tile([1, O], f32)
            nc.sync.dma_start(prow[:, :], P_d[:, :])
            krow = const.tile([1, O], f32)
            nc.sync.dma_start(krow[:, :], K_d[:, :])
            urow = const.tile([1, O], f32)
            nc.sync.dma_start(urow[:, :], U_d[:, :])
            pku = const.tile([1, O], f32)
            nc.vector.tensor_mul(pku[:, :], prow[:, :], krow[:, :])
            nc.vector.tensor_mul(pku[:, :], pku[:, :], urow[:, :])
            nc.vector.tensor_add(pku[:, :], pku[:, :], b2row[:, :])
            BIAS2 = const.tile([128, O], f32)
            nc.gpsimd.partition_broadcast(BIAS2[:, :], pku[:, :])

            degB = const.tile([128, NBLK * NBT], f32)
            nc.sync.dma_start(degB[:, :], degB_d[:, :])
            dinvB = const.tile([128, NBLK * NBT], f32)
            nc.vector.reciprocal(dinvB[:, :], degB[:, :])
            nc.scalar.sqrt(dinvB[:, :], dinvB[:, :])

            degS = const.tile([128, NT], f32)
            nc.sync.dma_start(degS[:, :], degS_d[:, :])
            dinvS = const.tile([128, NT], f32)
            nc.vector.reciprocal(dinvS[:, :], degS[:, :])
            nc.scalar.sqrt(dinvS[:, :], dinvS[:, :])

            idxS = const.tile([128, ntok // 16], i16)
            nc.sync.dma_start(idxS[:, :], idx_d[:, :])
            dloc = const.tile([128, nktot], f32)
            nc.sync.dma_start(dloc[:, :], dloc_d[:, :])

            agg = const.tile([128, NT, H], f32)
            nc.vector.memset(agg[:, :, :], 0.0)

            # ---------------- layer-1 message table: tab1_b = dinv*(x@W1)
            def build_table1(b):
                nodes_b = min(BLK, N - b * BLK)
                for s0 in range(0, nodes_b, SLAB):
                    w = min(SLAB, nodes_b - s0)
                    xs = xpool.tile([F, SLAB], f32, tag="xs")
                    nc.sync.dma_start(xs[:, :w],
                                      xT_d[:, b * BLK + s0: b * BLK + s0 + w])
                    st = t1pool.tile([128, _cdiv(SLAB, 128), H], f32, tag="t1st")
                    nfull = 0
                    for j0 in range(0, w, 128):
                        m = min(128, w - j0)
                        jt = (s0 + j0) // 128  # node-tile idx within block
                        ps = ps_bld.tile([128, H], f32, tag="psb")
                        nc.tensor.matmul(ps[:m, :], lhsT=xs[:, j0:j0 + m],
                                         rhs=W1s[:, :], start=True, stop=True)
                        nc.scalar.activation(
                            st[:m, j0 // 128, :], ps[:m, :], ACTF.Copy,
                            scale=dinvB[:m, b * NBT + jt: b * NBT + jt + 1])
                        if m == 128:
                            nfull += 1
                    # store staged tiles to DRAM
                    if nfull:
                        dst_ap = tab1[b][s0:s0 + nfull * 128, :].rearrange(
                            "(j p) f -> p j f", p=128)
                        nc.sync.dma_start(dst_ap, st[:, :nfull, :])
                    if nfull * 128 < w:  # ragged tail tile of the block
                        m = w - nfull * 128
                        nc.sync.dma_start(
                            tab1[b][s0 + nfull * 128: s0 + w, :],
                            st[:m, nfull, :])

            for b in range(NBLK):
                build_table1(b)

            # ---------------- gather + segment-sum matmul for one layer
            def seg_layer(table_aps, width):
                """table_aps[b] = AP of block b's message rows [rows, width]."""
                for b in range(NBLK):
                    kt_in_blk = blk_kt_base[b + 1] - blk_kt_base[b]
                    if kt_in_blk == 0:
                        continue
                    # gather chunks
                    gtiles = []
                    for ci in range(_cdiv(kt_in_blk, CH_KT)):
                        kts = min(CH_KT, kt_in_blk - ci * CH_KT)
                        g = gpool.tile([128, CH_KT, width], f32, tag="g")
                        tok0 = (blk_kt_base[b] + ci * CH_KT) * 128
                        nc.gpsimd.dma_gather(
                            g[:, :kts, :], table_aps[b],
                            idxS[:, tok0 // 16: (tok0 + kts * 128) // 16],
                            num_idxs=kts * 128, num_idxs_reg=kts * 128,
                            elem_size=width)
                        gtiles.append(g)
                    # consume
                    kk = 0
                    for t in range(NT):
                        nkt = int(nk[b, t])
                        if nkt == 0:
                            continue
                        ps = ps_seg.tile([128, width], f32, tag="pss")
                        for k in range(nkt):
                            ci, sl = divmod(kk + k, CH_KT)
                            kglob = blk_kt_base[b] + kk + k
                            S = spool.tile([128, 128], f32, tag="s")
                            nc.vector.tensor_scalar(
                                out=S[:, :], in0=IOTA[:, :],
                                scalar1=dloc[:, kglob:kglob + 1],
                                scalar2=None, op0=ALU.is_equal)
                            nc.tensor.matmul(ps[:, :], lhsT=S[:, :],
                                             rhs=gtiles[ci][:, sl, :],
                                             start=(k == 0), stop=(k == nkt - 1))
                        nc.vector.tensor_add(agg[:, t, :width],
                                             agg[:, t, :width], ps[:, :])
                        kk += nkt

            # ---------------- layer 1
            tab1_aps = [tab1[b][:, :] for b in range(NBLK)]
            seg_layer(tab1_aps, H)

            # post: h1 = relu(dinv*agg + b1); t2 = dinv*(h1@W2) padded
            def staged_store(dram, stile, grp, nt_in_grp, width):
                """store staging tile rows [grp*STG .. ) handling ragged tail"""
                t0 = grp * STG
                nfull = 0
                for tt in range(nt_in_grp):
                    if (t0 + tt) * 128 + 128 <= SHARD:
                        nfull += 1
                if nfull:
                    dst = dram[t0 * 128: t0 * 128 + nfull * 128, :].rearrange(
                        "(j p) f -> p j f", p=128)
                    nc.sync.dma_start(dst, stile[:, :nfull, :width])
                if nfull < nt_in_grp:
                    nc.sync.dma_start(
                        dram[(t0 + nfull) * 128: SHARD, :],
                        stile[:LASTV, nfull, :width])

            for grp in range(_cdiv(NT, STG)):
                nt_in_grp = min(STG, NT - grp * STG)
                st = post.tile([128, STG, HP], f32, tag="t2st")
                if HP > O:
                    nc.vector.memset(st[:, :, O:], 0.0)
                for tt in range(nt_in_grp):
                    t = grp * STG + tt
                    h1 = work.tile([128, H], f32, tag="h1")
                    nc.vector.scalar_tensor_tensor(
                        out=h1[:, :], in0=agg[:, t, :],
                        scalar=dinvS[:, t:t + 1], in1=BIAS1[:, :],
                        op0=ALU.mult, op1=ALU.add)
                    nc.scalar.activation(h1[:, :], h1[:, :], ACTF.Relu)
                    pst = ps_tr.tile([H, 128], f32, tag="pstr")
                    nc.tensor.transpose(pst[:, :], h1[:, :], ID[:, :])
                    h1t = work.tile([H, 128], f32, tag="h1t")
                    nc.scalar.copy(h1t[:, :], pst[:, :])
                    ps2 = ps_t2.tile([128, O], f32, tag="pst2")
                    nc.tensor.matmul(ps2[:, :], lhsT=h1t[:, :], rhs=W2s[:, :],
                                     start=True, stop=True)
                    nc.scalar.activation(st[:, tt, :O], ps2[:, :], ACTF.Copy,
                                         scale=dinvS[:, t:t + 1])
                staged_store(t2loc, st, grp, nt_in_grp, HP)

            # ---------------- exchange layer-2 table
            nc.gpsimd.collective_compute(
                "AllGather", mybir.AluOpType.bypass,
                replica_groups=[list(range(C))],
                ins=[t2loc[:, :].opt()],
                outs=[tab2[:, :].opt()])

            # ---------------- layer 2
            nc.vector.memset(agg[:, :, :], 0.0)
            tab2_aps = [tab2[b * BLK: b * BLK + min(BLK, N - b * BLK), :]
                        for b in range(NBLK)]
            seg_layer(tab2_aps, HP)

            # post: y = relu(dinv*agg + b2 + pku); out = log_softmax(y)
            for grp in range(_cdiv(NT, STG)):
                nt_in_grp = min(STG, NT - grp * STG)
                st = opool.tile([128, STG, O], f32, tag="ost")
                for tt in range(nt_in_grp):
                    t = grp * STG + tt
                    y = work.tile([128, O], f32, tag="y")
                    nc.vector.scalar_tensor_tensor(
                        out=y[:, :], in0=agg[:, t, :O],
                        scalar=dinvS[:, t:t + 1], in1=BIAS2[:, :],
                        op0=ALU.mult, op1=ALU.add)
                    nc.scalar.activation(y[:, :], y[:, :], ACTF.Relu)
                    nmax = work.tile([128, 1], f32, tag="nmax")
                    nc.vector.tensor_reduce(nmax[:, :], y[:, :],
                                            axis=mybir.AxisListType.X,
                                            op=ALU.max, negate=True)
                    ex = work.tile([128, O], f32, tag="ex")
                    esum = work.tile([128, 1], f32, tag="esum")
                    nc.scalar.activation(ex[:, :], y[:, :], ACTF.Exp,
                                         bias=nmax[:, :], scale=1.0,
                                         accum_out=esum[:, :])
                    lsum = work.tile([128, 1], f32, tag="lsum")
                    nc.scalar.activation(lsum[:, :], esum[:, :], ACTF.Ln)
                    nc.vector.tensor_scalar(
                        out=st[:, tt, :], in0=y[:, :], scalar1=nmax[:, :],
                        scalar2=lsum[:, :], op0=ALU.add, op1=ALU.subtract)
                staged_store(out_d, st, grp, nt_in_grp, O)

    nc.compile()
    return nc


# ------------------------------------------------------------------ entry --
def prepare_and_run(inputs, cfg=None, trace=False, **run_kwargs):
    """Preprocess, build, run on 8 cores.  Returns (out, BassKernelResults)."""
    from concourse.bass_utils import run_bass_kernel_spmd

    cfg = cfg or CFG()
    x = np.asarray(inputs["x"], dtype=np.float32)
    edge_index = np.asarray(inputs["edge_index"])
    W1 = np.asarray(inputs["W1"], dtype=np.float32)
    b1 = np.asarray(inputs["b1"], dtype=np.float32)
    W2 = np.asarray(inputs["W2"], dtype=np.float32)
    b2 = np.asarray(inputs["b2"], dtype=np.float32)
    P = np.asarray(inputs["P"], dtype=np.float32)
    K = np.asarray(inputs["K"], dtype=np.float32)
    U = np.asarray(inputs["U"], dtype=np.float32)

    deg, nk, per_core = _preprocess(edge_index, cfg)
    degB, degS = _wrap_deg(deg, cfg)
    xT = np.ascontiguousarray(x.T)

    nc = _build(cfg, nk)

    in_maps = []
    for c in range(cfg.C):
        in_maps.append({
            "xT": xT,
            "degB": degB,
            "degS": np.ascontiguousarray(degS[c]),
            "idx": per_core[c]["idx"],
            "dloc": per_core[c]["dloc"],
            "W1": W1, "W2": W2,
            "b1": b1.reshape(1, -1), "b2": b2.reshape(1, -1),
            "P": P.reshape(1, -1), "K": K.reshape(1, -1),
            "U": U.reshape(1, -1),
        })

    res = run_bass_kernel_spmd(nc, in_maps, core_ids=list(range(cfg.C)),
                               trace=trace, **run_kwargs)
    out = np.concatenate([res.results[c]["out"] for c in range(cfg.C)], axis=0)
    return out.astype(np.float32), res


def kernel(**inputs):
    out, _ = prepare_and_run(inputs)
    return out


if __name__ == "__main__":
    import reference

    inputs = {k: np.asarray(v) for k, v in reference.setup_inputs().items()}
    got = kernel(**inputs)
    want = np.asarray(reference.reference(**inputs))
    err = np.abs(got - want).max() / max(np.abs(want).max(), 1e-9)
    print("rel err:", err)
